# revision 1
# baseline (speedup 1.0000x reference)
"""MoE layer (top-2 of 8 experts) on 8 TRN2 NeuronCores.

Strategy:
  Phase 1 (device, data-parallel): each core computes gate logits
      logitsT = gate_w.T @ x_shard.T for B/8 tokens (fp32 matmul).
  Host: softmax + top-2 + renormalized weights (the routing / sharding
      decision), build per-expert token index lists, pad to a common
      capacity C (multiple of the token block).
  Phase 2 (device, expert-parallel): core e runs its expert's FFN over
      the tokens routed to it: y = (relu(x@W1+b1)@W2 + b2) * w_token.
      bf16 matmuls, fp32 PSUM accumulation, weights SBUF-resident.
  Host: scatter-add the two scaled contributions per token.
"""

import numpy as np
import ml_dtypes

import concourse.mybir as mybir
import concourse.tile as tile
from concourse import bacc
from concourse.bass_utils import run_bass_kernel_spmd

P = 128
N_CORES = 8
CB = 256  # phase-2 token block
BF16 = mybir.dt.bfloat16
F32 = mybir.dt.float32
_bf16_np = ml_dtypes.bfloat16

_build_cache = {}


def _build_gate(D, E, T):
    """Per-core gate matmul: logitsT[E, T] = gate_w[D, E].T @ xT[D, T]."""
    nc = bacc.Bacc(None, target_bir_lowering=False)
    xT = nc.dram_tensor("xT", [D, T], F32, kind="ExternalInput")
    gw = nc.dram_tensor("gw", [D, E], F32, kind="ExternalInput")
    logitsT = nc.dram_tensor("logitsT", [E, T], F32, kind="ExternalOutput")
    DO = D // P
    NT = 512
    xT_r = xT.rearrange("(do p) t -> p do t", p=P)
    with tile.TileContext(nc) as tc:
        with (
            tc.tile_pool(name="sb", bufs=2) as sb,
            tc.tile_pool(name="consts", bufs=1) as cp,
            tc.tile_pool(name="xp", bufs=2) as xp,
            tc.tile_pool(name="ps", bufs=2, space="PSUM") as ps,
        ):
            gw_sb = cp.tile([P, DO, E], F32, tag="gw")
            nc.sync.dma_start(gw_sb[:], gw.rearrange("(do p) e -> p do e", p=P))
            for tt in range(T // NT):
                # per-(token-tile, d-tile) x chunks (256KB) so the first
                # matmul starts as soon as the first chunk lands
                xdi = []
                for di in range(DO):
                    xt = xp.tile([P, NT], F32, tag=f"x{di}", name=f"x{di}")
                    eng = nc.sync if di % 2 == 0 else nc.scalar
                    eng.dma_start(xt[:], xT_r[:, di, tt * NT:(tt + 1) * NT])
                    xdi.append(xt)
                pt = ps.tile([E, NT], F32, tag="pt")
                for di in range(DO):
                    nc.tensor.matmul(
                        pt[:],
                        gw_sb[:, di],
                        xdi[di][:],
                        start=(di == 0),
                        stop=(di == DO - 1),
                    )
                ot = sb.tile([E, NT], F32, tag="ot")
                nc.vector.tensor_copy(ot[:], pt[:])
                nc.sync.dma_start(logitsT[:, tt * NT:(tt + 1) * NT], ot[:])
    nc.finalize()
    return nc


def _build_expert(D, H, O, C):
    """Per-core expert FFN over C (padded) routed tokens.

    y[C, O] = (relu(x @ W1 + b1) @ W2 + b2) * w_token[:, None]
    computed as hT = W1.T-slices @ xT (keeps H on partitions), then
    y = hT-slices.T @ W2 (tokens back on partitions). No transposes on
    device: xT / b1 / wt come host-prearranged.
    """
    nc = bacc.Bacc(None, target_bir_lowering=False)
    xT = nc.dram_tensor("xT", [D, C], BF16, kind="ExternalInput")
    w1 = nc.dram_tensor("w1", [D, H], BF16, kind="ExternalInput")
    w2 = nc.dram_tensor("w2", [H, O], BF16, kind="ExternalInput")
    b1 = nc.dram_tensor("b1", [P, H // P], F32, kind="ExternalInput")
    b2 = nc.dram_tensor("b2", [P, O], F32, kind="ExternalInput")
    wt = nc.dram_tensor("wt", [P, C // P], F32, kind="ExternalInput")
    y = nc.dram_tensor("y", [C, O], F32, kind="ExternalOutput")
    DO, HO = D // P, H // P
    OO = O // 512
    # token blocks of CB, trailing 128-block if C % CB != 0
    starts = []
    pos = 0
    while pos < C:
        cb = CB if C - pos >= CB else P
        starts.append((pos, cb))
        pos += cb
    # chunk the weight loads so the first matmuls start after ~1MB of DMA;
    # the first two W1 chunks are half-size so L1 starts even earlier
    HC = 4                   # h-tiles (of 128) per W2 weight chunk
    NWC = HO // HC           # number of W2 weight chunks
    w1_chunks = [(0, 2), (2, 2)] + [(h, 4) for h in range(4, HO, 4)]
    w1_of_hi = {}            # hi -> (chunk index, offset within chunk)
    for ci, (h0, nh) in enumerate(w1_chunks):
        for j in range(nh):
            w1_of_hi[h0 + j] = (ci, j)
    HG = 8                   # h-tiles per hT group tile (finer L2 deps)
    NHG = HO // HG
    y_r = y.rearrange("(n p) o -> p n o", p=P)
    w1_r = w1.rearrange("(do p) h -> p do h", p=P)
    w2_r = w2.rearrange("(ho p) o -> p ho o", p=P)
    with tile.TileContext(nc) as tc:
        with (
            tc.tile_pool(name="wpool", bufs=1) as wp,
            tc.tile_pool(name="xpool", bufs=3) as xp,
            tc.tile_pool(name="hpool", bufs=2) as hp,
            tc.tile_pool(name="opool", bufs=4) as op,
            tc.tile_pool(name="hps", bufs=4, space="PSUM") as hps,
            tc.tile_pool(name="yps", bufs=3, space="PSUM") as yps,
        ):
            xT_r = xT.rearrange("(do p) c -> p do c", p=P)
            # startup-critical DMAs: W1 chunks in consumption order on the
            # sync ring (L1 of block 0 chases W1's delivery); block-0 x,
            # W2 chunk 0 and biases on the scalar HWDGE ring.
            x0_sb = xp.tile([P, DO, CB], BF16, tag="x")
            nc.scalar.dma_start(x0_sb[:, :, :starts[0][1]], xT_r[:, :, 0:starts[0][1]])
            w1c = [wp.tile([P, DO, nh * P], BF16, tag=f"w1_{k}", name=f"w1_{k}")
                   for k, (h0, nh) in enumerate(w1_chunks)]
            w2c = [wp.tile([P, HC, O], BF16, tag=f"w2_{k}", name=f"w2_{k}") for k in range(NWC)]
            for k, (h0, nh) in enumerate(w1_chunks):
                nc.sync.dma_start(w1c[k][:], w1_r[:, :, h0 * P:(h0 + nh) * P])
            b1_sb = wp.tile([P, HO], F32, tag="b1")
            nc.scalar.dma_start(b1_sb[:], b1[:])
            nc.scalar.dma_start(w2c[0][:], w2_r[:, 0:HC])
            b2_sb = wp.tile([P, O], F32, tag="b2")
            nc.scalar.dma_start(b2_sb[:], b2[:])
            wt_sb = wp.tile([P, C // P], F32, tag="wt")
            nc.scalar.dma_start(wt_sb[:], wt[:])

            # W2 chunks 1.. are paced behind block-0 relus so they don't
            # race the critical W1 stream during startup
            w2_load_after = {
                4 * k: [(w2c[k], w2_r[:, k * HC:(k + 1) * HC])]
                for k in range(1, NWC)
            }
            for blk, (n0, cb) in enumerate(starts):
                if blk == 0:
                    x_sb = x0_sb[:, :, :cb]
                else:
                    x_sb = xp.tile([P, DO, CB], BF16, tag="x", name="x_sb")[:, :, :cb]
                    nc.sync.dma_start(x_sb[:], xT_r[:, :, n0:n0 + cb])
                hgs = [hp.tile([P, HG, CB], BF16, tag=f"h{g}", name=f"h{g}")[:, :, :cb]
                       for g in range(NHG)]
                for hi in range(HO):
                    ph = hps.tile([P, CB], F32, tag="ph", name="ph")[:, :cb]
                    ci, off = w1_of_hi[hi]
                    for di in range(DO):
                        nc.tensor.matmul(
                            ph[:],
                            w1c[ci][:, di, off * P:(off + 1) * P],
                            x_sb[:, di],
                            start=(di == 0),
                            stop=(di == DO - 1),
                        )
                    act = nc.scalar.activation(
                        hgs[hi // HG][:, hi % HG], ph[:],
                        mybir.ActivationFunctionType.Relu,
                        bias=b1_sb[:, hi:hi + 1],
                    )
                    if blk == 0 and hi in w2_load_after:
                        # W2 chunk k streams only after L1 consumed W1 chunk
                        # k, so it never races the critical W1 delivery
                        for w2t, w2src in w2_load_after[hi]:
                            dma = nc.scalar.dma_start(w2t[:], w2src)
                            tile.add_dep_helper(
                                dma.ins, act.ins,
                                reason="pace late load behind W1 consumption",
                            )
                for ct in range(cb // P):
                    # hi outer / ot inner: both ot matmuls share the same
                    # stationary hT slice, halving LDWEIGHTS pressure
                    yps_ct = [yps.tile([P, 512], F32, tag="yp", name="yp")
                              for _ in range(OO)]
                    for hi in range(HO):
                        for ot in range(OO):
                            nc.tensor.matmul(
                                yps_ct[ot][:],
                                hgs[hi // HG][:, hi % HG, ct * P:(ct + 1) * P],
                                w2c[hi // HC][:, hi % HC, ot * 512:(ot + 1) * 512],
                                start=(hi == 0),
                                stop=(hi == HO - 1),
                            )
                    for ot in range(OO):
                        o_sb = op.tile([P, 512], F32, tag="o")
                        nc.vector.tensor_add(
                            o_sb[:], yps_ct[ot][:], b2_sb[:, ot * 512:(ot + 1) * 512]
                        )
                        n_idx = n0 // P + ct
                        nc.vector.tensor_scalar_mul(
                            o_sb[:], o_sb[:], wt_sb[:, n_idx:n_idx + 1]
                        )
                        nc.sync.dma_start(
                            y_r[:, n_idx, ot * 512:(ot + 1) * 512], o_sb[:]
                        )
    nc.finalize()
    return nc


def kernel(x, W1, b1, W2, b2, gate_w, gate_b):
    x = np.ascontiguousarray(x, dtype=np.float32)
    W1 = np.asarray(W1, dtype=np.float32)
    b1 = np.asarray(b1, dtype=np.float32)
    W2 = np.asarray(W2, dtype=np.float32)
    b2 = np.asarray(b2, dtype=np.float32)
    gate_w = np.ascontiguousarray(gate_w, dtype=np.float32)
    gate_b = np.asarray(gate_b, dtype=np.float32)

    B, D = x.shape
    E, _, H = W1.shape
    O = W2.shape[2]
    assert E == N_CORES and B % (N_CORES * 512) == 0 and D % P == 0
    T = B // N_CORES
    core_ids = list(range(N_CORES))

    # ---- Phase 1: gate logits on device (data-parallel over tokens) ----
    key = ("gate", D, E, T)
    if key not in _build_cache:
        _build_cache[key] = _build_gate(D, E, T)
    nc_gate = _build_cache[key]
    in_maps = [
        {"xT": np.ascontiguousarray(x[i * T:(i + 1) * T].T), "gw": gate_w}
        for i in range(N_CORES)
    ]
    res = run_bass_kernel_spmd(nc_gate, in_maps, core_ids=core_ids)
    logits = np.concatenate(
        [res.results[i]["logitsT"].T for i in range(N_CORES)], axis=0
    ) + gate_b[None, :]

    # ---- Host: top-2 routing (the expert-parallel sharding decision) ----
    lg = logits.astype(np.float64)
    lg -= lg.max(axis=1, keepdims=True)
    probs = np.exp(lg)
    probs /= probs.sum(axis=1, keepdims=True)
    order = np.argsort(-probs, axis=1, kind="stable")[:, :2]
    p_top = np.take_along_axis(probs, order, axis=1)
    w_top = p_top / p_top.sum(axis=1, keepdims=True)  # [B, 2]

    idx_e, wt_e = [], []
    for e in range(E):
        m0 = order[:, 0] == e
        m1 = order[:, 1] == e
        sel = m0 | m1
        idx = np.nonzero(sel)[0]
        w = np.where(m0[sel], w_top[sel, 0], w_top[sel, 1]).astype(np.float32)
        idx_e.append(idx)
        wt_e.append(w)
    max_count = max(len(i) for i in idx_e)
    C = max(CB, ((max_count + P - 1) // P) * P)

    # ---- Phase 2: expert FFN on device (expert-parallel) ----
    key = ("expert", D, H, O, C)
    if key not in _build_cache:
        _build_cache[key] = _build_expert(D, H, O, C)
    nc_exp = _build_cache[key]

    in_maps = []
    for e in range(E):
        n_e = len(idx_e[e])
        xT_pad = np.zeros((D, C), dtype=_bf16_np)
        xT_pad[:, :n_e] = x[idx_e[e]].T.astype(_bf16_np)
        wt_pad = np.zeros(C, dtype=np.float32)
        wt_pad[:n_e] = wt_e[e]
        in_maps.append({
            "xT": xT_pad,
            "w1": W1[e].astype(_bf16_np),
            "w2": W2[e].astype(_bf16_np),
            "b1": np.ascontiguousarray(b1[e].reshape(H // P, P).T),
            "b2": np.ascontiguousarray(np.broadcast_to(b2[e], (P, O))),
            "wt": np.ascontiguousarray(wt_pad.reshape(C // P, P).T),
        })
    res = run_bass_kernel_spmd(nc_exp, in_maps, core_ids=core_ids)

    # ---- Host: un-permute and combine the two expert contributions ----
    out = np.zeros((B, O), dtype=np.float32)
    for e in range(E):
        n_e = len(idx_e[e])
        if n_e:
            out[idx_e[e]] += res.results[e]["y"][:n_e]
    return out



# revision 4
# speedup vs baseline: 1.1335x; 1.1335x over previous
"""MoE layer (top-2 of 8 experts) on 8 TRN2 NeuronCores.

Strategy (single device launch):
  Host: gate logits (tiny 8192x1024x8 sgemm), softmax + top-2 +
      renormalized weights, and the expert-parallel sharding decision.
  Device (one SPMD launch, 8 cores): each core runs 2 "slots"; a slot
      is (expert e, half of H) and processes all tokens routed to e:
      partial_y = relu(x @ W1[e][:, half] + b1) @ W2[e][half, :] * w_tok.
      Half-experts are assigned to slots sorted-balanced (big halves to
      slot 0, small to slot 1) so per-core work is ~Sum n_e/8 instead of
      max_e n_e. bf16 matmuls, fp32 PSUM.
  Host: sum the two H-halves and scatter-add the two scaled expert
      contributions per token (+ w-weighted b2 correction).
"""

import numpy as np
import ml_dtypes

import concourse.mybir as mybir
import concourse.tile as tile
from concourse import bacc
from concourse.bass_utils import run_bass_kernel_spmd

P = 128
N_CORES = 8
NS = 2          # slots (half-experts) per core
CB = 512        # token block
BF16 = mybir.dt.bfloat16
F32 = mybir.dt.float32
_bf16_np = ml_dtypes.bfloat16

_build_cache = {}


def _blocks(cap):
    """CB-sized token blocks with trailing 128-blocks."""
    out, pos = [], 0
    while pos < cap:
        cb = CB if cap - pos >= CB else P
        out.append((pos, cb))
        pos += cb
    return out


def _build_moe(D, HQ, O, caps):
    """Per-core program: NS slots, slot s = one (expert, H-half) over
    caps[s] padded routed tokens.

    Layer 1 keeps H on partitions (hT = W1-half.T-slices @ xT), layer 2
    puts tokens back on partitions (y = hT-slices.T @ W2-half). Weights
    arrive host-prearranged in SBUF layout; x/b1/wt host-prearranged.
    """
    DO, HO, OO = D // P, HQ // P, O // 512
    CT = sum(caps)
    nc = bacc.Bacc(None, target_bir_lowering=False)
    xT = nc.dram_tensor("xT", [D, CT], BF16, kind="ExternalInput")
    w1 = nc.dram_tensor("w1", [P, NS, DO, HQ], BF16, kind="ExternalInput")
    w2 = nc.dram_tensor("w2", [P, NS, HO, O], BF16, kind="ExternalInput")
    b1 = nc.dram_tensor("b1", [P, NS * HO], F32, kind="ExternalInput")
    wt = nc.dram_tensor("wt", [P, CT // P], F32, kind="ExternalInput")
    y = nc.dram_tensor("y", [CT, O], F32, kind="ExternalOutput")
    xT_r = xT.rearrange("(do p) c -> p do c", p=P)
    y_r = y.rearrange("(n p) o -> p n o", p=P)
    with tile.TileContext(nc) as tc:
        with (
            tc.tile_pool(name="wp", bufs=1) as wp,
            tc.tile_pool(name="xp", bufs=3) as xp,
            tc.tile_pool(name="hp", bufs=2) as hp,
            tc.tile_pool(name="op", bufs=4) as op,
            tc.tile_pool(name="hps", bufs=3, space="PSUM") as hps,
            tc.tile_pool(name="yps", bufs=4, space="PSUM") as yps,
        ):
            # weights stream on the scalar HWDGE ring in consumption
            # order; slot 0 chunked so block-0 matmuls chase the stream
            b1_sb = wp.tile([P, NS * HO], F32, tag="b1")
            nc.scalar.dma_start(b1_sb[:], b1[:])
            wt_sb = wp.tile([P, CT // P], F32, tag="wt")
            nc.scalar.dma_start(wt_sb[:], wt[:])
            w1_sb = [wp.tile([P, DO, HQ], BF16, tag=f"w1_{s}", name=f"w1_{s}")
                     for s in range(NS)]
            w2_sb = [wp.tile([P, HO, O], BF16, tag=f"w2_{s}", name=f"w2_{s}")
                     for s in range(NS)]
            HC = HQ // 4
            for k in range(4):
                nc.scalar.dma_start(
                    w1_sb[0][:, :, k * HC:(k + 1) * HC],
                    w1[:, 0, :, k * HC:(k + 1) * HC],
                )
            for k in range(2):
                nc.scalar.dma_start(
                    w2_sb[0][:, k * HO // 2:(k + 1) * HO // 2],
                    w2[:, 0, k * HO // 2:(k + 1) * HO // 2],
                )
            for s in range(1, NS):
                nc.scalar.dma_start(w1_sb[s][:], w1[:, s])
                nc.scalar.dma_start(w2_sb[s][:], w2[:, s])

            off = 0
            for s in range(NS):
                for (n0, cb) in _blocks(caps[s]):
                    g0 = off + n0
                    x_sb = xp.tile([P, DO, CB], BF16, tag="x", name="x_sb")[:, :, :cb]
                    nc.sync.dma_start(x_sb[:], xT_r[:, :, g0:g0 + cb])
                    hT = hp.tile([P, HO, CB], BF16, tag="h", name="hT")[:, :, :cb]
                    for hi in range(HO):
                        ph = hps.tile([P, CB], F32, tag="ph", name="ph")[:, :cb]
                        for di in range(DO):
                            nc.tensor.matmul(
                                ph[:],
                                w1_sb[s][:, di, hi * P:(hi + 1) * P],
                                x_sb[:, di],
                                start=(di == 0),
                                stop=(di == DO - 1),
                            )
                        nc.scalar.activation(
                            hT[:, hi], ph[:],
                            mybir.ActivationFunctionType.Relu,
                            bias=b1_sb[:, s * HO + hi:s * HO + hi + 1],
                        )
                    for ct in range(cb // P):
                        # hi outer / ot inner: both ot matmuls share the
                        # same stationary hT slice
                        yts = [yps.tile([P, 512], F32, tag="yp", name=f"yp{ot}")
                               for ot in range(OO)]
                        for hi in range(HO):
                            for ot in range(OO):
                                nc.tensor.matmul(
                                    yts[ot][:],
                                    hT[:, hi, ct * P:(ct + 1) * P],
                                    w2_sb[s][:, hi, ot * 512:(ot + 1) * 512],
                                    start=(hi == 0),
                                    stop=(hi == HO - 1),
                                )
                        ncol = g0 // P + ct
                        for ot in range(OO):
                            o_sb = op.tile([P, 512], F32, tag="o")
                            nc.vector.tensor_scalar_mul(
                                o_sb[:], yts[ot][:], wt_sb[:, ncol:ncol + 1]
                            )
                            nc.sync.dma_start(
                                y_r[:, ncol, ot * 512:(ot + 1) * 512], o_sb[:]
                            )
                off += caps[s]
    nc.finalize()
    return nc


def _pad128(n):
    return max(P, ((n + P - 1) // P) * P)


def kernel(x, W1, b1, W2, b2, gate_w, gate_b):
    x = np.ascontiguousarray(x, dtype=np.float32)
    W1 = np.asarray(W1, dtype=np.float32)
    b1 = np.asarray(b1, dtype=np.float32)
    W2 = np.asarray(W2, dtype=np.float32)
    b2 = np.asarray(b2, dtype=np.float32)
    gate_w = np.ascontiguousarray(gate_w, dtype=np.float32)
    gate_b = np.asarray(gate_b, dtype=np.float32)

    B, D = x.shape
    E, _, H = W1.shape
    O = W2.shape[2]
    HQ = H // NS
    HO = HQ // P
    assert E == N_CORES and D % P == 0 and H % (NS * P) == 0

    # ---- Host: gating + top-2 routing (the sharding decision) ----
    lg = x.astype(np.float64) @ gate_w.astype(np.float64) + gate_b
    lg -= lg.max(axis=1, keepdims=True)
    probs = np.exp(lg)
    probs /= probs.sum(axis=1, keepdims=True)
    order = np.argsort(-probs, axis=1, kind="stable")[:, :2]
    p_top = np.take_along_axis(probs, order, axis=1)
    w_top = (p_top / p_top.sum(axis=1, keepdims=True)).astype(np.float32)

    idx_e, wt_e = [], []
    for e in range(E):
        m0 = order[:, 0] == e
        m1 = order[:, 1] == e
        sel = m0 | m1
        idx = np.nonzero(sel)[0]
        w = np.where(m0[sel], w_top[sel, 0], w_top[sel, 1]).astype(np.float32)
        idx_e.append(idx)
        wt_e.append(w)

    # ---- Balanced slot assignment: NS half-experts per core ----
    units = sorted(
        [(len(idx_e[e]), e, q) for e in range(E) for q in range(NS)],
        key=lambda t: (-t[0], t[1], t[2]),
    )
    groups = [units[p * N_CORES:(p + 1) * N_CORES] for p in range(NS)]
    caps = tuple(_pad128(max(u[0] for u in g)) for g in groups)
    CT = sum(caps)

    key = ("moe", D, HQ, O, caps)
    if key not in _build_cache:
        _build_cache[key] = _build_moe(D, HQ, O, caps)
    nc = _build_cache[key]

    # ---- Build per-core inputs ----
    x_bf = x.astype(_bf16_np)
    xTe = {e: np.ascontiguousarray(x_bf[idx_e[e]].T) for e in range(E)}
    W1_bf = W1.astype(_bf16_np)
    W2_bf = W2.astype(_bf16_np)
    in_maps = []
    for c in range(N_CORES):
        slots = [groups[p][c] for p in range(NS)]
        xT = np.zeros((D, CT), dtype=_bf16_np)
        w1h = np.empty((P, NS, D // P, HQ), dtype=_bf16_np)
        w2h = np.empty((P, NS, HO, O), dtype=_bf16_np)
        b1h = np.zeros((P, NS * HO), dtype=np.float32)
        wth = np.zeros((P, CT // P), dtype=np.float32)
        off = 0
        for s, (n_u, e, q) in enumerate(slots):
            hsl = slice(q * HQ, (q + 1) * HQ)
            xT[:, off:off + n_u] = xTe[e]
            w1h[:, s] = W1_bf[e][:, hsl].reshape(D // P, P, HQ).transpose(1, 0, 2)
            w2h[:, s] = W2_bf[e][hsl].reshape(HO, P, O).transpose(1, 0, 2)
            b1h[:, s * HO:(s + 1) * HO] = b1[e][hsl].reshape(HO, P).T
            wpad = np.zeros(caps[s], dtype=np.float32)
            wpad[:n_u] = wt_e[e]
            wth[:, off // P:(off + caps[s]) // P] = wpad.reshape(-1, P).T
            off += caps[s]
        in_maps.append({
            "xT": xT,
            "w1": np.ascontiguousarray(w1h),
            "w2": np.ascontiguousarray(w2h),
            "b1": b1h,
            "wt": wth,
        })

    res = run_bass_kernel_spmd(nc, in_maps, core_ids=list(range(N_CORES)))

    # ---- Host: combine H-halves / experts, add gated b2 ----
    out = np.zeros((B, O), dtype=np.float32)
    for c in range(N_CORES):
        yc = res.results[c]["y"]
        off = 0
        for s in range(NS):
            n_u, e, q = groups[s][c]
            if n_u:
                out[idx_e[e]] += yc[off:off + n_u]
            off += caps[s]
    if np.any(b2):
        out += w_top[:, 0, None] * b2[order[:, 0]]
        out += w_top[:, 1, None] * b2[order[:, 1]]
    return out


# revision 7
# speedup vs baseline: 1.1442x; 1.0094x over previous
"""MoE layer (top-2 of 8 experts) on 8 TRN2 NeuronCores.

Strategy (single device launch):
  Host: gate logits (tiny 8192x1024x8 sgemm), softmax + top-2 +
      renormalized weights, and the expert-parallel sharding decision.
  Device (one SPMD launch, 8 cores): each core runs 2 "slots"; a slot
      is (expert e, half of H) and processes all tokens routed to e:
      partial_y = relu(x @ W1[e][:, half] + b1) @ W2[e][half, :] * w_tok.
      Half-experts are assigned to slots sorted-balanced (big halves to
      one slot group, small to the other) so per-core work is ~sum n_e/8
      instead of max_e n_e. bf16 matmuls, fp32 PSUM.
  Host: sum the two H-halves and scatter-add the two scaled expert
      contributions per token (+ w-weighted b2 correction).

Startup-critical path: the first slot's W1 arrives in chunk tiles (first
chunk 512KB, ahead of b1/wt on the scalar ring) and block 0's x arrives
in per-d-tile chunks on the sync ring, so the first matmul needs only
~640KB of DMA. The tail-less slot runs first so the program drains on a
128-token block whose two output DMAs go on different rings.
"""

import numpy as np
import ml_dtypes

import concourse.mybir as mybir
import concourse.tile as tile
from concourse import bacc
from concourse.bass_utils import run_bass_kernel_spmd

P = 128
N_CORES = 8
NS = 2          # slots (half-experts) per core
CB = 512        # token block
BF16 = mybir.dt.bfloat16
F32 = mybir.dt.float32
_bf16_np = ml_dtypes.bfloat16

_build_cache = {}


def _blocks(cap):
    """CB-sized token blocks with trailing 128-blocks."""
    out, pos = [], 0
    while pos < cap:
        cb = CB if cap - pos >= CB else P
        out.append((pos, cb))
        pos += cb
    return out


def _build_moe(D, HQ, O, caps):
    """Per-core program: NS slots, slot s = one (expert, H-half) over
    caps[s] padded routed tokens.

    Layer 1 keeps H on partitions (hT = W1-half.T-slices @ xT), layer 2
    puts tokens back on partitions (y = hT-slices.T @ W2-half). Weights
    arrive host-prearranged in SBUF layout; x/b1/wt host-prearranged.
    """
    DO, HO, OO = D // P, HQ // P, O // 512
    CT = sum(caps)
    nc = bacc.Bacc(None, target_bir_lowering=False)
    xT = nc.dram_tensor("xT", [D, CT], BF16, kind="ExternalInput")
    w1 = nc.dram_tensor("w1", [P, NS, DO, HQ], BF16, kind="ExternalInput")
    w2 = nc.dram_tensor("w2", [P, NS, HO, O], BF16, kind="ExternalInput")
    b1 = nc.dram_tensor("b1", [P, NS * HO], F32, kind="ExternalInput")
    wt = nc.dram_tensor("wt", [P, CT // P], F32, kind="ExternalInput")
    y = nc.dram_tensor("y", [CT, O], F32, kind="ExternalOutput")
    xT_r = xT.rearrange("(do p) c -> p do c", p=P)
    y_r = y.rearrange("(n p) o -> p n o", p=P)
    # first slot's W1 h-column chunks (first one small: block-0 matmuls
    # start after ~512KB) and W2 halves; later slots load whole
    W1CH = [(0, 2 * P), (2 * P, 6 * P)] + [
        (h0, 8 * P) for h0 in range(8 * P, HQ, 8 * P)
    ]
    n_ybr = sum(1 for c in caps for _ in _blocks(c))  # total blocks
    with tile.TileContext(nc) as tc:
        with (
            tc.tile_pool(name="wp", bufs=1) as wp,
            tc.tile_pool(name="xp", bufs=2) as xp,
            tc.tile_pool(name="x0p", bufs=1) as x0p,
            tc.tile_pool(name="hp", bufs=2) as hp,
            tc.tile_pool(name="op", bufs=4) as op,
            tc.tile_pool(name="hps", bufs=3, space="PSUM") as hps,
            tc.tile_pool(name="yps", bufs=4, space="PSUM") as yps,
        ):
            # --- weight / const streams (scalar HWDGE ring, in order) ---
            w1f = []            # first slot W1: (h0, tile of [P, DO, nh])
            for k, (h0, nh) in enumerate(W1CH):
                t = wp.tile([P, DO, nh], BF16, tag=f"w1f{k}", name=f"w1f{k}")
                nc.scalar.dma_start(t[:], w1[:, 0, :, h0:h0 + nh])
                w1f.append((h0, nh, t))
                if k == 0:
                    b1_sb = wp.tile([P, NS * HO], F32, tag="b1", name="b1_sb")
                    nc.scalar.dma_start(b1_sb[:], b1[:])
                    wt_sb = wp.tile([P, CT // P], F32, tag="wt", name="wt_sb")
                    nc.scalar.dma_start(wt_sb[:], wt[:])
            w2f = []            # first slot W2 in two half tiles
            for k in range(2):
                t = wp.tile([P, HO // 2, O], BF16, tag=f"w2f{k}", name=f"w2f{k}")
                nc.scalar.dma_start(t[:], w2[:, 0, k * HO // 2:(k + 1) * HO // 2])
                w2f.append(t)
            w1r = [None]        # later slots: whole tiles
            w2r = [None]
            for s in range(1, NS):
                t1 = wp.tile([P, DO, HQ], BF16, tag=f"w1_{s}", name=f"w1_{s}")
                nc.scalar.dma_start(t1[:], w1[:, s])
                t2 = wp.tile([P, HO, O], BF16, tag=f"w2_{s}", name=f"w2_{s}")
                nc.scalar.dma_start(t2[:], w2[:, s])
                w1r.append(t1)
                w2r.append(t2)

            def w1_slice(s, di, hi):
                if s > 0:
                    return w1r[s][:, di, hi * P:(hi + 1) * P]
                for (h0, nh, t) in w1f:
                    if h0 <= hi * P < h0 + nh:
                        return t[:, di, hi * P - h0:(hi + 1) * P - h0]
                raise AssertionError

            def w2_slice(s, hi, ot):
                if s > 0:
                    return w2r[s][:, hi, ot * 512:(ot + 1) * 512]
                return w2f[hi // (HO // 2)][:, hi % (HO // 2), ot * 512:(ot + 1) * 512]

            # --- main loop ---
            off = 0
            blk_idx = 0
            for s in range(NS):
                for (n0, cb) in _blocks(caps[s]):
                    g0 = off + n0
                    first = blk_idx == 0
                    last = blk_idx == n_ybr - 1
                    if first:
                        # per-d-tile x chunks: first matmul waits on 1/8
                        xds = []
                        for di in range(DO):
                            xt = x0p.tile([P, CB], BF16, tag=f"x0d{di}",
                                          name=f"x0d{di}")[:, :cb]
                            nc.sync.dma_start(xt[:], xT_r[:, di, g0:g0 + cb])
                            xds.append(xt)
                        x_of = lambda di: xds[di]
                    else:
                        x_sb = xp.tile([P, DO, CB], BF16, tag="x",
                                       name="x_sb")[:, :, :cb]
                        nc.sync.dma_start(x_sb[:], xT_r[:, :, g0:g0 + cb])
                        x_of = lambda di: x_sb[:, di]
                    hT = hp.tile([P, HO, CB], BF16, tag="h", name="hT")[:, :, :cb]
                    for hi in range(HO):
                        ph = hps.tile([P, CB], F32, tag="ph", name="ph")[:, :cb]
                        for di in range(DO):
                            nc.tensor.matmul(
                                ph[:],
                                w1_slice(s, di, hi),
                                x_of(di),
                                start=(di == 0),
                                stop=(di == DO - 1),
                            )
                        nc.scalar.activation(
                            hT[:, hi], ph[:],
                            mybir.ActivationFunctionType.Relu,
                            bias=b1_sb[:, s * HO + hi:s * HO + hi + 1],
                        )
                    for ct in range(cb // P):
                        # hi outer / ot inner: both ot matmuls share the
                        # same stationary hT slice
                        yts = [yps.tile([P, 512], F32, tag="yp", name=f"yp{ot}")
                               for ot in range(OO)]
                        for hi in range(HO):
                            for ot in range(OO):
                                nc.tensor.matmul(
                                    yts[ot][:],
                                    hT[:, hi, ct * P:(ct + 1) * P],
                                    w2_slice(s, hi, ot),
                                    start=(hi == 0),
                                    stop=(hi == HO - 1),
                                )
                        ncol = g0 // P + ct
                        for ot in range(OO):
                            o_sb = op.tile([P, 512], F32, tag="o")
                            nc.vector.tensor_scalar_mul(
                                o_sb[:], yts[ot][:], wt_sb[:, ncol:ncol + 1]
                            )
                            # split the drain of the very last block
                            eng = nc.scalar if (last and ot == 1) else nc.sync
                            eng.dma_start(
                                y_r[:, ncol, ot * 512:(ot + 1) * 512], o_sb[:]
                            )
                    blk_idx += 1
                off += caps[s]
    nc.finalize()
    return nc


def _pad128(n):
    return max(P, ((n + P - 1) // P) * P)


def kernel(x, W1, b1, W2, b2, gate_w, gate_b):
    x = np.ascontiguousarray(x, dtype=np.float32)
    W1 = np.asarray(W1, dtype=np.float32)
    b1 = np.asarray(b1, dtype=np.float32)
    W2 = np.asarray(W2, dtype=np.float32)
    b2 = np.asarray(b2, dtype=np.float32)
    gate_w = np.ascontiguousarray(gate_w, dtype=np.float32)
    gate_b = np.asarray(gate_b, dtype=np.float32)

    B, D = x.shape
    E, _, H = W1.shape
    O = W2.shape[2]
    HQ = H // NS
    HO = HQ // P
    assert E == N_CORES and D % P == 0 and H % (NS * P) == 0

    # ---- Host: gating + top-2 routing (the sharding decision) ----
    lg = x.astype(np.float64) @ gate_w.astype(np.float64) + gate_b
    lg -= lg.max(axis=1, keepdims=True)
    probs = np.exp(lg)
    probs /= probs.sum(axis=1, keepdims=True)
    order = np.argsort(-probs, axis=1, kind="stable")[:, :2]
    p_top = np.take_along_axis(probs, order, axis=1)
    w_top = (p_top / p_top.sum(axis=1, keepdims=True)).astype(np.float32)

    idx_e, wt_e = [], []
    for e in range(E):
        m0 = order[:, 0] == e
        m1 = order[:, 1] == e
        sel = m0 | m1
        idx = np.nonzero(sel)[0]
        w = np.where(m0[sel], w_top[sel, 0], w_top[sel, 1]).astype(np.float32)
        idx_e.append(idx)
        wt_e.append(w)

    # ---- Balanced slot assignment: NS half-experts per core ----
    units = sorted(
        [(len(idx_e[e]), e, q) for e in range(E) for q in range(NS)],
        key=lambda t: (-t[0], t[1], t[2]),
    )
    groups = [units[p * N_CORES:(p + 1) * N_CORES] for p in range(NS)]
    caps = [_pad128(max(u[0] for u in g)) for g in groups]
    # process tail-less slots first so the program drains on a 128-block
    proc = sorted(range(NS), key=lambda s: (caps[s] % CB != 0, -caps[s]))
    groups = [groups[s] for s in proc]
    caps = tuple(caps[s] for s in proc)
    CT = sum(caps)

    key = ("moe", D, HQ, O, caps)
    if key not in _build_cache:
        _build_cache[key] = _build_moe(D, HQ, O, caps)
    nc = _build_cache[key]

    # ---- Build per-core inputs ----
    x_bf = x.astype(_bf16_np)
    xTe = {e: np.ascontiguousarray(x_bf[idx_e[e]].T) for e in range(E)}
    W1_bf = W1.astype(_bf16_np)
    W2_bf = W2.astype(_bf16_np)
    in_maps = []
    for c in range(N_CORES):
        slots = [groups[p][c] for p in range(NS)]
        xT = np.zeros((D, CT), dtype=_bf16_np)
        w1h = np.empty((P, NS, D // P, HQ), dtype=_bf16_np)
        w2h = np.empty((P, NS, HO, O), dtype=_bf16_np)
        b1h = np.zeros((P, NS * HO), dtype=np.float32)
        wth = np.zeros((P, CT // P), dtype=np.float32)
        off = 0
        for s, (n_u, e, q) in enumerate(slots):
            hsl = slice(q * HQ, (q + 1) * HQ)
            xT[:, off:off + n_u] = xTe[e]
            w1h[:, s] = W1_bf[e][:, hsl].reshape(D // P, P, HQ).transpose(1, 0, 2)
            w2h[:, s] = W2_bf[e][hsl].reshape(HO, P, O).transpose(1, 0, 2)
            b1h[:, s * HO:(s + 1) * HO] = b1[e][hsl].reshape(HO, P).T
            wpad = np.zeros(caps[s], dtype=np.float32)
            wpad[:n_u] = wt_e[e]
            wth[:, off // P:(off + caps[s]) // P] = wpad.reshape(-1, P).T
            off += caps[s]
        in_maps.append({
            "xT": xT,
            "w1": np.ascontiguousarray(w1h),
            "w2": np.ascontiguousarray(w2h),
            "b1": b1h,
            "wt": wth,
        })

    res = run_bass_kernel_spmd(nc, in_maps, core_ids=list(range(N_CORES)))

    # ---- Host: combine H-halves / experts, add gated b2 ----
    out = np.zeros((B, O), dtype=np.float32)
    for c in range(N_CORES):
        yc = res.results[c]["y"]
        off = 0
        for s in range(NS):
            n_u, e, q = groups[s][c]
            if n_u:
                out[idx_e[e]] += yc[off:off + n_u]
            off += caps[s]
    if np.any(b2):
        out += w_top[:, 0, None] * b2[order[:, 0]]
        out += w_top[:, 1, None] * b2[order[:, 1]]
    return out
